# revision 2
# baseline (speedup 1.0000x reference)
"""GQA attention (B=4, S=1024, H=4096, 32 q heads / 8 kv heads, rotary) on 8 trn2 cores.

Sharding: DP4 x TP2. Core c = 2*b + j handles batch b with kv-head half j:
  - column-parallel wq/wk/wv (16 q heads / 4 kv heads per core)
  - row-parallel wo -> partial [S, H] outputs, host sums core pairs.

All-bf16 matmul pipeline (1 cyc/row on the PE), fp32 PSUM accumulate,
everything SBUF-resident (no DRAM spills). Measured ~734 us/core NEFF
exec (neuron-profile), PE >94% busy at the 216 ns/matmul N=512 floor.

  - All inputs host-swizzled partition-major so every DMA is >=8 KiB
    contiguous per partition (2 KiB strided rows run ~1/3 line rate).
  - xT resident in SBUF bf16 (64 KiB/partition); K then V then 16 fused
    Q-blocks; v is computed directly in natural [t, d] layout
    (lhsT = x slice, rhs = wv) - no PE transposes.
  - Rope via host-precomputed sin/cos maps; the pair-partner partition
    swap is a 64-partition SBUF-to-SBUF DMA.
  - Softmax: exp on ACT (PSUM -> bf16 SBUF); tb-axis denominator
    reduction on DVE (3 strided adds at 2x bf16 rate); single K=128
    matmul for the cross-partition sum; 1/denom broadcast by a K=1
    matmul + reciprocal_approx_fast on the [128, 512] broadcast.
  - Software pipeline: iter cb emits proj(cb) groups, then the 16
    single-MM scores of block cb-1 with the tail groups (denom / bcast /
    av / normalize) of block cb-2 interleaved BETWEEN them - PSUM
    accumulation groups must stay contiguous on HW (interleaving foreign
    matmuls INSIDE a group corrupts results; found empirically, CoreSim
    does not model it), and the 1:2 alternation keeps the PE fed while
    ACT drains exp and DVE runs the reduction tree.
  - Phase 3 (out = oT.T @ wo) double-buffers wo strips on the sync DMA
    ring while outputs stream back at 2 KiB/partition.
"""

import numpy as np

B = 4
S = 1024
H = 4096
D = 128
HQ = 32
HKV = 8
G = 4
NCORES = 8
QC = 2048  # q cols per core
KC = 512  # k cols per core
VC = 512  # v cols per core
COH = 2048  # wo rows per core
ROPE_BASE = 10000.0

_CACHE = {}


def _build(reps=1):
    import concourse.tile as tile
    from concourse import bacc, mybir

    fp32 = mybir.dt.float32
    f32r = mybir.dt.float32r
    bf16 = mybir.dt.bfloat16

    nc = bacc.Bacc(None, target_bir_lowering=False)

    NKO = H // 128  # 32 contraction tiles

    # All inputs are host-swizzled to partition-major layouts so every DMA
    # is >=8 KiB contiguous per partition (2 KiB strided rows run at ~1/3
    # of HBM line rate).
    xT_r = nc.dram_tensor("xT", [128, NKO, S], bf16, kind="ExternalInput")
    wq_r = nc.dram_tensor("wq", [128, QC // 128, NKO, 128], bf16,
                          kind="ExternalInput")
    wk_r = nc.dram_tensor("wk", [128, KC // 128, NKO, 128], bf16,
                          kind="ExternalInput")
    wv_r = nc.dram_tensor("wv", [128, NKO, VC], bf16, kind="ExternalInput")
    wo_r = nc.dram_tensor("wo", [128, 8, 16, 512], bf16, kind="ExternalInput")
    aq_d = nc.dram_tensor("ropeAq", [D, S], fp32, kind="ExternalInput")
    bq_d = nc.dram_tensor("ropeBq", [D, S], fp32, kind="ExternalInput")
    ak_d = nc.dram_tensor("ropeAk", [D, S], fp32, kind="ExternalInput")
    bk_d = nc.dram_tensor("ropeBk", [D, S], fp32, kind="ExternalInput")
    out_d = nc.dram_tensor("out", [S, H], fp32, kind="ExternalOutput")

    out_r = out_d.rearrange("(tb p) h -> tb p h", p=128)  # [8, 128, 4096]

    with tile.TileContext(nc) as tc, nc.allow_low_precision(
        reason="bf16 matmul pipeline"
    ):
      for _rep in range(reps):
        with (
            tc.tile_pool(name="persist", bufs=1) as persist,
            tc.tile_pool(name="konst", bufs=1) as konst,
        ):
            xsb = persist.tile([128, NKO, S], bf16)  # 64 KiB/part
            kT = persist.tile([128, HKV // 2, S], bf16)  # [128, 4, 1024]
            v = persist.tile([128, S // 128, VC], bf16)  # [128, 8, 512]
            oT = persist.tile([128, 16, S], bf16)  # 32 KiB/part
            maps = {}
            for nm, dram in (
                ("Aq", aq_d), ("Bq", bq_d), ("Ak", ak_d), ("Bk", bk_d)
            ):
                mt = persist.tile([128, S], fp32, name=nm)
                maps[nm] = mt

            ones_f = konst.tile([128, 128], fp32)
            nc.vector.memset(ones_f[:], 1.0)
            ones_b = konst.tile([128, 128], bf16)
            nc.vector.tensor_copy(ones_b[:], ones_f[:])

            # x chunks and rope maps are DMA'd after the first weight
            # tile inside the pool scope below (HWDGE ring is FIFO per
            # engine: the first proj group must not queue behind 10 MB).
            def stage_inputs():
                for ch in range(4):
                    nc.sync.dma_start(
                        xsb[:, ch * 8 : (ch + 1) * 8, :],
                        xT_r[:, ch * 8 : (ch + 1) * 8, :],
                    )
                for nm, dram in (
                    ("Aq", aq_d), ("Bq", bq_d), ("Ak", ak_d), ("Bk", bk_d)
                ):
                    nc.sync.dma_start(maps[nm][:], dram[:])

            with (
                tc.tile_pool(name="wt", bufs=2) as wpool,
                tc.tile_pool(name="ev", bufs=2) as epool,
                tc.tile_pool(name="ps1", bufs=2, space="PSUM") as ps1,
                tc.tile_pool(name="pssc", bufs=2, space="PSUM") as pssc,
                tc.tile_pool(name="psden", bufs=1, space="PSUM") as psden,
                tc.tile_pool(name="psbc", bufs=1, space="PSUM") as psbc,
                tc.tile_pool(name="pso", bufs=2, space="PSUM") as pso,
            ):

                def load_wt(w_r, cb, eng=None):
                    wt = wpool.tile([128, NKO, 128], bf16, tag="wt", name="wt")
                    (eng or nc.sync).dma_start(wt[:], w_r[:, cb])
                    return wt

                def rope_evict(ps, Am, Bm, out_ap, th):
                    """psum [128,512] -> rope -> out_ap (any dtype)."""
                    ts_ = slice(th * 512, th * 512 + 512)
                    raw = epool.tile([128, 512], fp32, tag="raw", name="raw")
                    nc.scalar.copy(raw[:], ps[:])
                    t1 = epool.tile([128, 512], fp32, tag="t1", name="t1")
                    nc.vector.tensor_mul(t1[:], raw[:], Am[:, ts_])
                    sw = epool.tile([128, 512], fp32, tag="sw", name="sw")
                    nc.sync.dma_start(sw[0:64, :], raw[64:128, :])
                    nc.sync.dma_start(sw[64:128, :], raw[0:64, :])
                    t2 = epool.tile([128, 512], fp32, tag="t2", name="t2")
                    nc.vector.tensor_mul(t2[:], sw[:], Bm[:, ts_])
                    nc.vector.tensor_add(out_ap, t1[:], t2[:])

                def emit_proj_half(wt, th, hook=None):
                    """32-ko accumulation into a fresh psum tile; returns it.
                    hook(j) is called after every 4th matmul (j=0..7)."""
                    ps = ps1.tile([128, 512], fp32, tag="ps1", name="ps")
                    for ko in range(NKO):
                        nc.tensor.matmul(
                            ps[:],
                            wt[:, ko, :],
                            xsb[:, ko, th * 512 : th * 512 + 512],
                            start=(ko == 0),
                            stop=(ko == NKO - 1),
                        )
                        if hook is not None and ko % 4 == 3:
                            hook(ko // 4)
                    return ps

                # ---------------- K projections + V ----------------
                wt_next = [load_wt(wk_r, 0)]
                stage_inputs()
                with tc.tile_pool(name="wvp", bufs=1) as wvpool:
                    for cb in range(4):
                        wt = wt_next[0]
                        if cb < 3:
                            wt_next[0] = load_wt(wk_r, cb + 1, nc.scalar)
                        else:
                            wv_sb = wvpool.tile(
                                [128, NKO, VC], bf16, name="wv_sb"
                            )
                            nc.scalar.dma_start(wv_sb[:], wv_r[:])
                        for th in range(2):
                            ps = emit_proj_half(wt, th)
                            rope_evict(
                                ps, maps["Ak"], maps["Bk"],
                                kT[:, cb, th * 512 : th * 512 + 512], th,
                            )

                    # V in natural [t, d] layout: lhsT = x slice (stationary)
                    for tb in range(8):
                        psv = ps1.tile([128, 512], fp32, tag="ps1", name="psv")
                        for ko in range(NKO):
                            nc.tensor.matmul(
                                psv[:],
                                xsb[:, ko, tb * 128 : (tb + 1) * 128],
                                wv_sb[:, ko, :],
                                start=(ko == 0),
                                stop=(ko == NKO - 1),
                            )
                        if tb == 0:
                            wt_next[0] = load_wt(wq_r, 0)
                        nc.scalar.copy(v[:, tb, :], psv[:])

                # ---------------- fused Q proj + attention ----------------
                fused_pools = (
                    tc.tile_pool(name="qt", bufs=2),
                    tc.tile_pool(name="ex", bufs=3),
                    tc.tile_pool(name="sm", bufs=2),
                    tc.tile_pool(name="tr", bufs=1),
                )
                qpool = fused_pools[0].__enter__()
                expool = fused_pools[1].__enter__()
                smpool = fused_pools[2].__enter__()
                trpool = fused_pools[3].__enter__()
                state = {}

                def attn_scores(cb, qt, sh, j):
                    """one scores matmul + exp for (cb, sh), tb=j."""
                    h = cb // 4
                    psc = pssc.tile([128, 512], fp32, tag="psc", name="psc")
                    nc.tensor.matmul(
                        psc[:],
                        kT[:, h, j * 128 : (j + 1) * 128],
                        qt[:, sh * 512 : sh * 512 + 512],
                        start=True,
                        stop=True,
                    )
                    nc.scalar.activation(
                        state[(cb, sh)][:, j], psc[:],
                        mybir.ActivationFunctionType.Exp,
                    )

                def tail_pieces(cb):
                    """Emitter callables for block cb's softmax tail; each is
                    one contiguous PE accumulation group (or pure DVE/ACT),
                    safe to interleave between other groups."""
                    h = cb // 4
                    ctx = {}

                    def denom(sh):
                        # tb-axis reduction on DVE (PE only does the final
                        # cross-partition sum): 3 strided adds, then 1 matmul.
                        expT = state[(cb, sh)]
                        h1 = trpool.tile([128, 4, 512], bf16, tag="h1", name="h1")
                        nc.vector.tensor_add(h1[:], expT[:, 0:4], expT[:, 4:8])
                        nc.vector.tensor_add(h1[:, 0:2], h1[:, 0:2], h1[:, 2:4])
                        h3 = trpool.tile([128, 512], bf16, tag="h3", name="h3")
                        nc.vector.tensor_add(h3[:], h1[:, 0], h1[:, 1])
                        pden = psden.tile([1, 512], fp32, tag="pd", name="pd")
                        nc.tensor.matmul(
                            pden[:], ones_b[:, 0:1], h3[:],
                            start=True, stop=True,
                        )
                        den = smpool.tile([1, 512], bf16, tag="den", name="den")
                        nc.scalar.copy(den[:], pden[:])
                        ctx[("den", sh)] = den

                    def bcast(sh):
                        pbc = psbc.tile([128, 512], fp32, tag="pb", name="pb")
                        nc.tensor.matmul(
                            pbc[:], ones_b[0:1, :], ctx[("den", sh)][:],
                            start=True, stop=True,
                        )
                        invb = smpool.tile(
                            [128, 512], fp32, tag="invb", name="invb"
                        )
                        nc.vector.reciprocal_approx_fast(invb[:], pbc[:])
                        ctx[("invb", sh)] = invb

                    def av(sh):
                        expT = state[(cb, sh)]
                        po = pso.tile([128, 512], fp32, tag="po", name="po")
                        for tb in range(8):
                            nc.tensor.matmul(
                                po[:],
                                v[:, tb, h * 128 : (h + 1) * 128],
                                expT[:, tb],
                                start=(tb == 0),
                                stop=(tb == 7),
                            )
                        ctx[("po", sh)] = po

                    def mul(sh):
                        ss = slice(sh * 512, sh * 512 + 512)
                        nc.vector.tensor_mul(
                            oT[:, cb, ss], ctx[("po", sh)][:],
                            ctx[("invb", sh)][:],
                        )
                        state.pop((cb, sh))

                    return [
                        lambda: denom(0), lambda: bcast(0),
                        lambda: denom(1), lambda: bcast(1),
                        lambda: av(0), lambda: mul(0),
                        lambda: av(1), lambda: mul(1),
                    ]

                def scores_block(cb, qt, tail):
                    """16 scores singles of block cb, with the (contiguous)
                    tail groups of an older block interleaved between them."""
                    for sh in range(2):
                        state[(cb, sh)] = expool.tile(
                            [128, 8, 512], bf16, tag="expT", name="expT"
                        )
                        for j in range(8):
                            attn_scores(cb, qt, sh, j)
                            if (sh * 8 + j) % 2 == 1 and tail:
                                tail.pop(0)()
                    while tail:
                        tail.pop(0)()

                qts = {}
                for cb in range(16):
                    wt = wt_next[0]
                    if cb < 15:
                        wt_next[0] = load_wt(wq_r, cb + 1)
                    qt = qpool.tile([128, S], bf16, tag="qt", name="qt")
                    for sh in range(2):
                        ps = emit_proj_half(wt, sh)
                        rope_evict(
                            ps, maps["Aq"], maps["Bq"],
                            qt[:, sh * 512 : sh * 512 + 512], sh,
                        )
                    qts[cb] = qt
                    if cb > 0:
                        scores_block(
                            cb - 1, qts[cb - 1],
                            tail_pieces(cb - 2) if cb > 1 else [],
                        )
                        del qts[cb - 1]
                scores_block(15, qts[15], tail_pieces(14))
                for piece in tail_pieces(15):
                    piece()
                for p in reversed(fused_pools):
                    p.__exit__(None, None, None)

            # ---------------- out = oT.T @ wo ----------------
            with (
                tc.tile_pool(name="wot", bufs=4) as wopool,
                tc.tile_pool(name="outp", bufs=3) as outpool,
                tc.tile_pool(name="psout", bufs=3, space="PSUM") as psout,
            ):
                def load_wo_strip(hh, half):
                    wot = wopool.tile(
                        [128, 8, 512], bf16, tag="wo", name="wo"
                    )
                    nc.sync.dma_start(
                        wot[:], wo_r[:, hh, half * 8 : (half + 1) * 8, :]
                    )
                    return wot

                wo_next = [load_wo_strip(0, 0), load_wo_strip(0, 1)]
                for hh in range(8):
                    hs = slice(hh * 512, hh * 512 + 512)
                    wotA, wotB = wo_next
                    for tb in range(8):
                        pso_ = psout.tile([128, 512], fp32, tag="pso", name="pso_")
                        for co in range(8):
                            nc.tensor.matmul(
                                pso_[:],
                                oT[:, co, tb * 128 : (tb + 1) * 128],
                                wotA[:, co, :],
                                start=(co == 0),
                                stop=False,
                            )
                        if tb == 0 and hh < 7:
                            wo_next[0] = load_wo_strip(hh + 1, 0)
                        if tb == 1 and hh < 7:
                            wo_next[1] = load_wo_strip(hh + 1, 1)
                        for co in range(8, 16):
                            nc.tensor.matmul(
                                pso_[:],
                                oT[:, co, tb * 128 : (tb + 1) * 128],
                                wotB[:, co - 8, :],
                                start=False,
                                stop=(co == 15),
                            )
                        ot = outpool.tile([128, 512], fp32, tag="ot", name="ot")
                        nc.scalar.copy(ot[:], pso_[:])
                        nc.sync.dma_start(out_r[tb, :, hs], ot[:])

    nc.compile()
    return nc


def _host_prep(x, wq, wk, wv, wo, start_pos):
    import ml_dtypes

    bf16 = ml_dtypes.bfloat16
    x = np.asarray(x, dtype=np.float32)
    wq = np.asarray(wq, dtype=np.float32)
    wk = np.asarray(wk, dtype=np.float32)
    wv = np.asarray(wv, dtype=np.float32)
    wo = np.asarray(wo, dtype=np.float32)
    sp = int(np.asarray(start_pos))

    perm = np.concatenate([np.arange(0, 128, 2), np.arange(1, 128, 2)])

    def permute_cols(w):
        n = w.shape[1]
        return np.ascontiguousarray(
            w.reshape(H, n // 128, 128)[:, :, perm].reshape(H, n)
        )

    inv_freq = 1.0 / (ROPE_BASE ** (np.arange(0, D, 2, dtype=np.float32) / D))
    t = np.arange(sp, sp + S, dtype=np.float32)
    freqs = t[None, :] * inv_freq[:, None]  # [64, S]
    sin, cos = np.sin(freqs), np.cos(freqs)
    A = np.concatenate([sin, sin], axis=0).astype(np.float32)  # [128, S]
    Bm = np.concatenate([-cos, cos], axis=0).astype(np.float32)
    scale = np.float32(1.0 / np.sqrt(np.float32(D)))
    maps = {
        "ropeAq": np.ascontiguousarray(A * scale),
        "ropeBq": np.ascontiguousarray(Bm * scale),
        "ropeAk": A,
        "ropeBk": Bm,
    }

    NKO = H // 128

    def sw_x(xh):  # [H, S] -> [128, NKO, S]
        return np.ascontiguousarray(
            xh.reshape(NKO, 128, S).swapaxes(0, 1)
        )

    def sw_w(w):  # [H, C] -> [128, C//128, NKO, 128]
        nblk = w.shape[1] // 128
        return np.ascontiguousarray(
            w.reshape(NKO, 128, nblk, 128).transpose(1, 2, 0, 3)
        )

    def sw_wv(w):  # [H, VC] -> [128, NKO, VC]
        return np.ascontiguousarray(w.reshape(NKO, 128, VC).swapaxes(0, 1))

    def sw_wo(w):  # [COH, H] -> [128, 8, 16, 512]
        return np.ascontiguousarray(
            w.reshape(16, 128, 8, 512).transpose(1, 2, 0, 3)
        )

    in_maps = []
    for c in range(NCORES):
        b, j = divmod(c, 2)
        im = {
            "xT": sw_x(x[b].T).astype(bf16),
            "wq": sw_w(permute_cols(wq[:, j * QC : (j + 1) * QC])).astype(bf16),
            "wk": sw_w(permute_cols(wk[:, j * KC : (j + 1) * KC])).astype(bf16),
            "wv": sw_wv(wv[:, j * VC : (j + 1) * VC]).astype(bf16),
            "wo": sw_wo(wo[j * COH : (j + 1) * COH, :]).astype(bf16),
        }
        im.update(maps)
        in_maps.append(im)
    return in_maps


def kernel(x, wq, wk, wv, wo, start_pos=0, _trace=False):
    from concourse.bass_utils import run_bass_kernel_spmd

    if "nc" not in _CACHE:
        _CACHE["nc"] = _build()
    nc = _CACHE["nc"]

    in_maps = _host_prep(x, wq, wk, wv, wo, start_pos)
    res = run_bass_kernel_spmd(nc, in_maps, core_ids=list(range(NCORES)), trace=_trace)
    _CACHE["last_result"] = res

    out = np.empty((B, S, H), dtype=np.float32)
    for b in range(B):
        out[b] = res.results[2 * b]["out"] + res.results[2 * b + 1]["out"]
    return out
